# revision 12
# baseline (speedup 1.0000x reference)
"""Trainium2 Bass kernel for nn_BertGenerationMoE (moe_routing).

Expert-parallel over 8 NeuronCores: core m owns experts {2m, 2m+1}.
Host side does routing/dispatch (gather + the torch-scatter col-0 sum +
layout transposes) — pure data movement; the device does all FLOPs:
per-expert  h1 = gelu(buf @ W1 + b1);  y = h1 @ W2 + b2 + buf;
LN(y) * g + b;  out[c>0] += out[0]  (the combine-einsum token add).

Matmuls run in float32r (fp32 data, full-rate PE mode, ~1e-4 rel err).

Shapes (hardcoded from the problem): S=256, L=128, H=512, I=2048, E=16,
C=S/E=16, 8 cores, 2 experts/core, 2048 rows (=C*L) per expert.
"""
import sys

sys.path.insert(0, "/opt/trn_rl_repo")

import numpy as np

import concourse.bass as bass
import concourse.tile as tile
from concourse import bacc, mybir
from concourse import bass_utils

F32 = mybir.dt.float32
F32R = mybir.dt.float32r
GELU = mybir.ActivationFunctionType.Gelu
IDENT = mybir.ActivationFunctionType.Identity
SQRT = mybir.ActivationFunctionType.Sqrt

S, L, H, I, E = 256, 128, 512, 2048, 16
NCORES = 8
EPL = E // NCORES          # experts per core = 2
C = S // E                 # capacity = 16
ROWS = C * L               # 2048 rows per expert
KC1 = H // 128             # 4  k-chunks of GEMM1
IC = I // 128              # 16 i-chunks
RC = 4                     # row chunks of 512
RSC = 4                    # 128-row subchunks per row chunk
LN_EPS = 1e-12

_CACHE = {}


def build_nc(num_iters=None, use_b1=False, use_b2=False, use_g=False, use_lb=False):
    """Build + compile the per-core Bass program. num_iters wraps the body in a
    For_i for steady-state timing; None = single shot."""
    key = (num_iters, use_b1, use_b2, use_g, use_lb)
    if key in _CACHE:
        return _CACHE[key]

    nc = bacc.Bacc("TRN2", target_bir_lowering=False, debug=False, num_devices=NCORES)

    xt = nc.dram_tensor("xt", [EPL, H, ROWS], F32, kind="ExternalInput").ap()
    xn = nc.dram_tensor("xn", [EPL, C, L, H], F32, kind="ExternalInput").ap()
    w1 = nc.dram_tensor("w1", [EPL, H, I], F32, kind="ExternalInput").ap()
    w2 = nc.dram_tensor("w2", [EPL, I, H], F32, kind="ExternalInput").ap()
    b1 = nc.dram_tensor("b1", [EPL, I], F32, kind="ExternalInput").ap()
    b2 = nc.dram_tensor("b2", [EPL, H], F32, kind="ExternalInput").ap()
    lg = nc.dram_tensor("lg", [EPL, H], F32, kind="ExternalInput").ap()
    lb = nc.dram_tensor("lb", [EPL, H], F32, kind="ExternalInput").ap()
    z = nc.dram_tensor("z", [EPL, C, L, H], F32, kind="ExternalOutput").ap()

    def bcast(src_1d):
        # [H] dram vector -> partition-broadcast AP [128, H]
        return bass.AP(tensor=src_1d.tensor, offset=src_1d.offset,
                       ap=[[0, 128]] + [list(p) for p in src_1d.ap])

    def body(tc):
        with (
            tc.tile_pool(name="pw1", bufs=2) as pw1,
            tc.tile_pool(name="pw2", bufs=1) as pw2,
            tc.tile_pool(name="pxt", bufs=3) as pxt,
            tc.tile_pool(name="ph1", bufs=1) as ph1,
            tc.tile_pool(name="pxn", bufs=3) as pxn,
            tc.tile_pool(name="py", bufs=4) as py,
            tc.tile_pool(name="py0", bufs=2) as py0,
            tc.tile_pool(name="pz", bufs=2) as pz,
            tc.tile_pool(name="pst", bufs=8) as pst,
            tc.tile_pool(name="psmall", bufs=2) as psmall,
            tc.tile_pool(name="ps1", bufs=4, space="PSUM") as ps1,
            tc.tile_pool(name="ps2", bufs=4, space="PSUM") as ps2,
        ):
            eps_t = pst.tile([128, 1], F32, tag="eps")
            nc.vector.memset(eps_t, LN_EPS)

            for e in range(EPL):
                w1t = pw1.tile([128, KC1, I], F32R, tag="w1")
                w1_src = w1[e].rearrange("(c p) i -> p c i", p=128).bitcast(F32R)
                for kc in range(KC1):
                    nc.scalar.dma_start(out=w1t[:, kc:kc + 1, :],
                                        in_=w1_src[:, kc:kc + 1, :])

                w2t = pw2.tile([128, IC, H], F32R, tag="w2")
                w2_src = w2[e].rearrange("(c p) h -> p c h", p=128).bitcast(F32R)
                for q in range(4):
                    nc.scalar.dma_start(out=w2t[:, 4 * q:4 * q + 4, :],
                                        in_=w2_src[:, 4 * q:4 * q + 4, :])

                if use_b1:
                    b1t = psmall.tile([128, IC], F32, tag="b1")
                    nc.scalar.dma_start(out=b1t, in_=b1[e].rearrange("(c p) -> p c", p=128))
                if use_b2:
                    b2t = psmall.tile([128, H], F32, tag="b2")
                    nc.scalar.dma_start(out=b2t, in_=bcast(b2[e]))
                if use_g:
                    lgt = psmall.tile([128, H], F32, tag="lg")
                    nc.scalar.dma_start(out=lgt, in_=bcast(lg[e]))
                if use_lb:
                    lbt = psmall.tile([128, H], F32, tag="lb")
                    nc.scalar.dma_start(out=lbt, in_=bcast(lb[e]))

                xt_src = xt[e].rearrange("(c p) r -> p c r", p=128).bitcast(F32R)
                y0 = py0.tile([128, H], F32, tag="y0")

                for rc in range(RC):
                    xtt = pxt.tile([128, KC1, 512], F32R, tag="xt")
                    nc.sync.dma_start(out=xtt, in_=xt_src[:, :, rc * 512:(rc + 1) * 512])

                    # residual rows for this row chunk (4 slots)
                    xnt = pxn.tile([128, RSC, H], F32, tag="xn")
                    nc.gpsimd.dma_start(
                        out=xnt,
                        in_=xn[e, rc * 4:(rc + 1) * 4].rearrange("c l h -> l c h"))

                    h1ts = []
                    for i in range(IC):
                        h1i = ph1.tile([128, 512], F32R, tag=f"h1_{i}")
                        h1ts.append(h1i)
                        p1 = ps1.tile([128, 512], F32, tag="ps1")
                        for kc in range(KC1):
                            nc.tensor.matmul(
                                p1, w1t[:, kc, i * 128:(i + 1) * 128], xtt[:, kc, :],
                                start=(kc == 0), stop=(kc == KC1 - 1))
                        if use_b1:
                            nc.scalar.activation(out=h1i, in_=p1, func=GELU,
                                                 bias=b1t[:, i:i + 1], scale=1.0)
                        else:
                            nc.scalar.activation(out=h1i, in_=p1, func=GELU)

                    zbuf = pz.tile([128, RSC, H], F32, tag="z")
                    for rsc in range(RSC):
                        slot = rc * 4 + rsc
                        p2 = ps2.tile([128, H], F32, tag="ps2")
                        for i in range(IC):
                            nc.tensor.matmul(
                                p2, h1ts[i][:, rsc * 128:(rsc + 1) * 128], w2t[:, i, :],
                                start=(i == 0), stop=(i == IC - 1))
                        # y = psum + residual (+ b2)
                        yt = py.tile([128, H], F32, tag="y")
                        nc.vector.tensor_add(yt, p2, xnt[:, rsc, :])
                        if use_b2:
                            nc.vector.tensor_add(yt, yt, b2t)
                        # LayerNorm stats
                        st = pst.tile([128, 6], F32, tag="st")
                        nc.vector.bn_stats(out=st, in_=yt)
                        mv = pst.tile([128, 2], F32, tag="mv")
                        nc.vector.bn_aggr(out=mv, in_=st)
                        rstd = pst.tile([128, 1], F32, tag="rstd")
                        nc.scalar.activation(out=rstd, in_=mv[:, 1:2], func=SQRT,
                                             bias=eps_t, scale=1.0)
                        nc.vector.reciprocal(out=rstd, in_=rstd)
                        m2 = pst.tile([128, 1], F32, tag="m2")
                        nc.vector.tensor_mul(m2, mv[:, 0:1], rstd)
                        nc.vector.tensor_scalar_mul(m2, m2, -1.0)
                        # apply LN (+ g, + b), write into output staging
                        if slot == 0:
                            nc.scalar.activation(out=y0, in_=yt, func=IDENT,
                                                 bias=m2, scale=rstd)
                            if use_g:
                                nc.vector.tensor_mul(y0, y0, lgt)
                            if use_lb:
                                nc.vector.tensor_add(y0, y0, lbt)
                            nc.vector.tensor_copy(zbuf[:, 0, :], y0)
                        else:
                            dst = zbuf[:, rsc, :]
                            nc.scalar.activation(out=dst, in_=yt, func=IDENT,
                                                 bias=m2, scale=rstd)
                            if use_g:
                                nc.vector.tensor_mul(dst, dst, lgt)
                            if use_lb:
                                nc.vector.tensor_add(dst, dst, lbt)
                            # combine: += LN-output of slot 0
                            nc.vector.tensor_add(dst, dst, y0)
                    nc.gpsimd.dma_start(
                        out=z[e, rc * 4:(rc + 1) * 4].rearrange("c l h -> l c h"),
                        in_=zbuf)

    with tile.TileContext(nc) as tc:
        if num_iters is None:
            body(tc)
        else:
            with tc.For_i(0, num_iters, 1):
                body(tc)

    nc.compile()
    _CACHE[key] = nc
    return nc


def _route(task_ids, hash_indices):
    """Token order per expert (reference cumsum rank order = seq order)."""
    dest = np.asarray(hash_indices)[np.asarray(task_ids)]
    toks = [np.nonzero(dest == e)[0] for e in range(E)]
    for e in range(E):
        assert len(toks[e]) == C, f"unbalanced routing: expert {e} got {len(toks[e])}"
    return toks


def kernel(**inputs):
    x = np.ascontiguousarray(np.asarray(inputs["x"], dtype=np.float32))
    task_ids = np.asarray(inputs["task_ids"], dtype=np.int32)
    hash_indices = np.asarray(inputs["hash_indices"], dtype=np.int32)
    W1 = np.asarray(inputs["W1"], dtype=np.float32)
    b1 = np.asarray(inputs["b1"], dtype=np.float32)
    W2 = np.asarray(inputs["W2"], dtype=np.float32)
    b2 = np.asarray(inputs["b2"], dtype=np.float32)
    ln_g = np.asarray(inputs["ln_g"], dtype=np.float32)
    ln_b = np.asarray(inputs["ln_b"], dtype=np.float32)

    toks = _route(task_ids, hash_indices)

    # dispatch: buf[e, 0] = sum of all routed seqs (torch scatter col-0 quirk),
    # buf[e, c>0] = seq with rank c
    xg = x[np.stack(toks)]                      # [E, C, L, H]
    buf = xg.copy()
    buf[:, 0] = xg.sum(axis=1)

    use_b1 = bool(np.any(b1))
    use_b2 = bool(np.any(b2))
    use_g = bool(np.any(ln_g != 1.0))
    use_lb = bool(np.any(ln_b))
    nc = build_nc(None, use_b1, use_b2, use_g, use_lb)

    in_maps = []
    for m in range(NCORES):
        sl = slice(EPL * m, EPL * (m + 1))
        bufm = buf[sl]                          # [EPL, C, L, H]
        flat = bufm.reshape(EPL, ROWS, H)
        in_maps.append({
            "xt": np.ascontiguousarray(flat.transpose(0, 2, 1)),
            "xn": np.ascontiguousarray(bufm),
            "w1": np.ascontiguousarray(W1[sl]),
            "w2": np.ascontiguousarray(W2[sl]),
            "b1": np.ascontiguousarray(b1[sl]),
            "b2": np.ascontiguousarray(b2[sl]),
            "lg": np.ascontiguousarray(ln_g[sl]),
            "lb": np.ascontiguousarray(ln_b[sl]),
        })

    res = bass_utils.run_bass_kernel_spmd(nc, in_maps, core_ids=list(range(NCORES)))

    out = np.empty((S, L, H), dtype=np.float32)
    for m in range(NCORES):
        zm = res.results[m]["z"]               # [EPL, C, L, H]
        for j in range(EPL):
            out[toks[EPL * m + j]] = zm[j]
    return out, task_ids.copy()


# revision 13
# speedup vs baseline: 2.5391x; 2.5391x over previous
"""Trainium2 Bass kernel for nn_BertGenerationMoE (moe_routing).

Expert-parallel over 8 NeuronCores: core m owns experts {2m, 2m+1}.
Host side does routing/dispatch (gather + the torch-scatter col-0 sum +
layout transposes) — pure data movement; the device does all FLOPs:
per-expert  h1 = gelu(buf @ W1 + b1);  y = h1 @ W2 + b2 + buf;
LN(y) * g + b;  out[c>0] += out[0]  (the combine-einsum token add).

Matmuls run in float32r (fp32 data, full-rate PE mode, ~1e-4 rel err).

Shapes (hardcoded from the problem): S=256, L=128, H=512, I=2048, E=16,
C=S/E=16, 8 cores, 2 experts/core, 2048 rows (=C*L) per expert.
"""
import sys

sys.path.insert(0, "/opt/trn_rl_repo")

import numpy as np

import concourse.bass as bass
import concourse.tile as tile
from concourse import bacc, mybir
from concourse import bass_utils

F32 = mybir.dt.float32
F32R = mybir.dt.float32r
GELU = mybir.ActivationFunctionType.Gelu
IDENT = mybir.ActivationFunctionType.Identity
SQRT = mybir.ActivationFunctionType.Sqrt

S, L, H, I, E = 256, 128, 512, 2048, 16
NCORES = 8
EPL = E // NCORES          # experts per core = 2
C = S // E                 # capacity = 16
ROWS = C * L               # 2048 rows per expert
KC1 = H // 128             # 4  k-chunks of GEMM1
IC = I // 128              # 16 i-chunks
RC = 4                     # row chunks of 512
RSC = 4                    # 128-row subchunks per row chunk
LN_EPS = 1e-12

_CACHE = {}


def build_nc(num_iters=None, use_b1=False, use_b2=False, use_g=False, use_lb=False):
    """Build + compile the per-core Bass program. num_iters wraps the body in a
    For_i for steady-state timing; None = single shot."""
    key = (num_iters, use_b1, use_b2, use_g, use_lb)
    if key in _CACHE:
        return _CACHE[key]

    nc = bacc.Bacc("TRN2", target_bir_lowering=False, debug=False, num_devices=NCORES)

    xt = nc.dram_tensor("xt", [EPL, H, ROWS], F32, kind="ExternalInput").ap()
    xn = nc.dram_tensor("xn", [EPL, C, L, H], F32, kind="ExternalInput").ap()
    w1 = nc.dram_tensor("w1", [EPL, H, I], F32, kind="ExternalInput").ap()
    w2 = nc.dram_tensor("w2", [EPL, I, H], F32, kind="ExternalInput").ap()
    b1 = nc.dram_tensor("b1", [EPL, I], F32, kind="ExternalInput").ap()
    b2 = nc.dram_tensor("b2", [EPL, H], F32, kind="ExternalInput").ap()
    lg = nc.dram_tensor("lg", [EPL, H], F32, kind="ExternalInput").ap()
    lb = nc.dram_tensor("lb", [EPL, H], F32, kind="ExternalInput").ap()
    z = nc.dram_tensor("z", [EPL, C, L, H], F32, kind="ExternalOutput").ap()

    def bcast(src_1d):
        # [H] dram vector -> partition-broadcast AP [128, H]
        return bass.AP(tensor=src_1d.tensor, offset=src_1d.offset,
                       ap=[[0, 128]] + [list(p) for p in src_1d.ap])

    def body(tc):
        with (
            tc.tile_pool(name="pw1", bufs=2) as pw1,
            tc.tile_pool(name="pw2", bufs=1) as pw2,
            tc.tile_pool(name="pxt", bufs=3) as pxt,
            tc.tile_pool(name="ph1", bufs=1) as ph1,
            tc.tile_pool(name="pxn", bufs=3) as pxn,
            tc.tile_pool(name="py", bufs=4) as py,
            tc.tile_pool(name="py0", bufs=2) as py0,
            tc.tile_pool(name="pz", bufs=2) as pz,
            tc.tile_pool(name="pst", bufs=8) as pst,
            tc.tile_pool(name="psmall", bufs=2) as psmall,
            tc.tile_pool(name="ps1", bufs=4, space="PSUM") as ps1,
            tc.tile_pool(name="ps2", bufs=4, space="PSUM") as ps2,
        ):
            eps_t = pst.tile([128, 1], F32, tag="eps")
            nc.vector.memset(eps_t, LN_EPS)

            for e in range(EPL):
                w1t = pw1.tile([128, KC1, I], F32R, tag="w1")
                w1_src = w1[e].rearrange("(c p) i -> p c i", p=128).bitcast(F32R)
                nc.scalar.dma_start(out=w1t[:, 0:2, :], in_=w1_src[:, 0:2, :])
                nc.scalar.dma_start(out=w1t[:, 2:4, :], in_=w1_src[:, 2:4, :])

                w2t = pw2.tile([128, IC, H], F32R, tag="w2")
                w2_src = w2[e].rearrange("(c p) h -> p c h", p=128).bitcast(F32R)
                nc.scalar.dma_start(out=w2t[:, 0:8, :], in_=w2_src[:, 0:8, :])
                nc.scalar.dma_start(out=w2t[:, 8:16, :], in_=w2_src[:, 8:16, :])

                if use_b1:
                    b1t = psmall.tile([128, IC], F32, tag="b1")
                    nc.scalar.dma_start(out=b1t, in_=b1[e].rearrange("(c p) -> p c", p=128))
                if use_b2:
                    b2t = psmall.tile([128, H], F32, tag="b2")
                    nc.scalar.dma_start(out=b2t, in_=bcast(b2[e]))
                if use_g:
                    lgt = psmall.tile([128, H], F32, tag="lg")
                    nc.scalar.dma_start(out=lgt, in_=bcast(lg[e]))
                if use_lb:
                    lbt = psmall.tile([128, H], F32, tag="lb")
                    nc.scalar.dma_start(out=lbt, in_=bcast(lb[e]))

                xt_src = xt[e].rearrange("(c p) r -> p c r", p=128).bitcast(F32R)
                y0 = py0.tile([128, H], F32, tag="y0")

                for rc in range(RC):
                    xtt = pxt.tile([128, KC1, 512], F32R, tag="xt")
                    nc.sync.dma_start(out=xtt, in_=xt_src[:, :, rc * 512:(rc + 1) * 512])

                    # residual rows for this row chunk (4 slots)
                    xnt = pxn.tile([128, RSC, H], F32, tag="xn")
                    nc.gpsimd.dma_start(
                        out=xnt,
                        in_=xn[e, rc * 4:(rc + 1) * 4].rearrange("c l h -> l c h"))

                    h1ts = []
                    for i in range(IC):
                        h1i = ph1.tile([128, 512], F32R, tag=f"h1_{i}")
                        h1ts.append(h1i)
                        p1 = ps1.tile([128, 512], F32, tag="ps1")
                        for kc in range(KC1):
                            nc.tensor.matmul(
                                p1, w1t[:, kc, i * 128:(i + 1) * 128], xtt[:, kc, :],
                                start=(kc == 0), stop=(kc == KC1 - 1))
                        if use_b1:
                            nc.scalar.activation(out=h1i, in_=p1, func=GELU,
                                                 bias=b1t[:, i:i + 1], scale=1.0)
                        else:
                            nc.scalar.activation(out=h1i, in_=p1, func=GELU)

                    zbuf = pz.tile([128, RSC, H], F32, tag="z")
                    for rsc in range(RSC):
                        slot = rc * 4 + rsc
                        p2 = ps2.tile([128, H], F32, tag="ps2")
                        for i in range(IC):
                            nc.tensor.matmul(
                                p2, h1ts[i][:, rsc * 128:(rsc + 1) * 128], w2t[:, i, :],
                                start=(i == 0), stop=(i == IC - 1))
                        # y = psum + residual (+ b2)
                        yt = py.tile([128, H], F32, tag="y")
                        nc.vector.tensor_add(yt, p2, xnt[:, rsc, :])
                        if use_b2:
                            nc.vector.tensor_add(yt, yt, b2t)
                        # LayerNorm stats
                        st = pst.tile([128, 6], F32, tag="st")
                        nc.vector.bn_stats(out=st, in_=yt)
                        mv = pst.tile([128, 2], F32, tag="mv")
                        nc.vector.bn_aggr(out=mv, in_=st)
                        rstd = pst.tile([128, 1], F32, tag="rstd")
                        nc.scalar.activation(out=rstd, in_=mv[:, 1:2], func=SQRT,
                                             bias=eps_t, scale=1.0)
                        nc.vector.reciprocal(out=rstd, in_=rstd)
                        m2 = pst.tile([128, 1], F32, tag="m2")
                        nc.vector.tensor_mul(m2, mv[:, 0:1], rstd)
                        nc.vector.tensor_scalar_mul(m2, m2, -1.0)
                        # apply LN (+ g, + b), write into output staging
                        if slot == 0:
                            nc.scalar.activation(out=y0, in_=yt, func=IDENT,
                                                 bias=m2, scale=rstd)
                            if use_g:
                                nc.vector.tensor_mul(y0, y0, lgt)
                            if use_lb:
                                nc.vector.tensor_add(y0, y0, lbt)
                            nc.vector.tensor_copy(zbuf[:, 0, :], y0)
                        else:
                            dst = zbuf[:, rsc, :]
                            nc.scalar.activation(out=dst, in_=yt, func=IDENT,
                                                 bias=m2, scale=rstd)
                            if use_g:
                                nc.vector.tensor_mul(dst, dst, lgt)
                            if use_lb:
                                nc.vector.tensor_add(dst, dst, lbt)
                            # combine: += LN-output of slot 0
                            nc.vector.tensor_add(dst, dst, y0)
                    nc.gpsimd.dma_start(
                        out=z[e, rc * 4:(rc + 1) * 4].rearrange("c l h -> l c h"),
                        in_=zbuf)

    with tile.TileContext(nc) as tc:
        if num_iters is None:
            body(tc)
        else:
            with tc.For_i(0, num_iters, 1):
                body(tc)

    nc.compile()
    _CACHE[key] = nc
    return nc


def _route(task_ids, hash_indices):
    """Token order per expert (reference cumsum rank order = seq order)."""
    dest = np.asarray(hash_indices)[np.asarray(task_ids)]
    toks = [np.nonzero(dest == e)[0] for e in range(E)]
    for e in range(E):
        assert len(toks[e]) == C, f"unbalanced routing: expert {e} got {len(toks[e])}"
    return toks


def kernel(**inputs):
    x = np.ascontiguousarray(np.asarray(inputs["x"], dtype=np.float32))
    task_ids = np.asarray(inputs["task_ids"], dtype=np.int32)
    hash_indices = np.asarray(inputs["hash_indices"], dtype=np.int32)
    W1 = np.asarray(inputs["W1"], dtype=np.float32)
    b1 = np.asarray(inputs["b1"], dtype=np.float32)
    W2 = np.asarray(inputs["W2"], dtype=np.float32)
    b2 = np.asarray(inputs["b2"], dtype=np.float32)
    ln_g = np.asarray(inputs["ln_g"], dtype=np.float32)
    ln_b = np.asarray(inputs["ln_b"], dtype=np.float32)

    toks = _route(task_ids, hash_indices)

    # dispatch: buf[e, 0] = sum of all routed seqs (torch scatter col-0 quirk),
    # buf[e, c>0] = seq with rank c
    xg = x[np.stack(toks)]                      # [E, C, L, H]
    buf = xg.copy()
    buf[:, 0] = xg.sum(axis=1)

    use_b1 = bool(np.any(b1))
    use_b2 = bool(np.any(b2))
    use_g = bool(np.any(ln_g != 1.0))
    use_lb = bool(np.any(ln_b))
    nc = build_nc(None, use_b1, use_b2, use_g, use_lb)

    in_maps = []
    for m in range(NCORES):
        sl = slice(EPL * m, EPL * (m + 1))
        bufm = buf[sl]                          # [EPL, C, L, H]
        flat = bufm.reshape(EPL, ROWS, H)
        in_maps.append({
            "xt": np.ascontiguousarray(flat.transpose(0, 2, 1)),
            "xn": np.ascontiguousarray(bufm),
            "w1": np.ascontiguousarray(W1[sl]),
            "w2": np.ascontiguousarray(W2[sl]),
            "b1": np.ascontiguousarray(b1[sl]),
            "b2": np.ascontiguousarray(b2[sl]),
            "lg": np.ascontiguousarray(ln_g[sl]),
            "lb": np.ascontiguousarray(ln_b[sl]),
        })

    res = bass_utils.run_bass_kernel_spmd(nc, in_maps, core_ids=list(range(NCORES)))

    out = np.empty((S, L, H), dtype=np.float32)
    for m in range(NCORES):
        zm = res.results[m]["z"]               # [EPL, C, L, H]
        for j in range(EPL):
            out[toks[EPL * m + j]] = zm[j]
    return out, task_ids.copy()
